# revision 10
# baseline (speedup 1.0000x reference)
"""Trainium2 Bass kernel for nn_DCLMBlock (B=4, S=2048, H=1536, 8 cores).

Sharding: data-parallel over batch (4) x sequence-halves (2) = 8 cores.
Core (b, h) computes output tokens [h*1024, (h+1)*1024) of batch b. All cores
run one SPMD program on [prefix | main] = 2048 tokens; first-half cores get a
zero prefix (contributing exactly the reference's causal zero padding),
second-half cores recompute the sequential prefix work (conv stacks, memory
scan inputs) from the real left context.

Layout: channels on partitions, tokens on the free dim ([128, chunk, t]).
GEMMs in bf16 with fp32 PSUM accumulation; residual stream fp32.
"""

import numpy as np
import ml_dtypes

import concourse.bacc as bacc
import concourse.mybir as mybir
from concourse import tile
from concourse.bass_utils import run_bass_kernel_spmd

F32 = mybir.dt.float32
BF16 = mybir.dt.bfloat16
AF = mybir.ActivationFunctionType
OP = mybir.AluOpType

P = 128
H, HI = 1536, 12
T, TM = 2048, 1024
NB, NBM = 4, 2
NH, HD, MEM, ST, NCH = 12, 128, 128, 64, 32
INNER = 6144
FI = 2 * INNER
STACK_DILS = (1, 2, 4, 8, 16, 32)
HEAD_DILS = ((1, 2, 4), (1, 1, 1), (4, 8, 16), (8, 16, 32), (32, 64, 128),
             (64, 128, 256), (256, 512, 1024), (1, 100, 200), (1, 500, 1000),
             (1, 1024, 2048), (3, 9, 27), (5, 25, 125))
MEM_HEADS = (6, 7, 8, 9)
EPS = 1e-6
BF = ml_dtypes.bfloat16


def build_program(gelu_func=AF.Gelu, dumps=()):
    nc = bacc.Bacc("TRN2", target_bir_lowering=False, debug=False)
    dp = nc.declare_dram_parameter

    d = {}
    d["xT"] = dp("xT", [H, T], F32, isOutput=False)
    for nm in ("nw1", "nw2", "nw3"):
        d[nm] = dp(nm, [H], F32, isOutput=False)
    d["csw"] = dp("csw", [P, 6 * HI * 4], F32, isOutput=False)
    d["csb"] = dp("csb", [P, 6 * HI], F32, isOutput=False)
    d["cpT"] = dp("cpT", [H, H], BF16, isOutput=False)
    d["cpb"] = dp("cpb", [P, HI], F32, isOutput=False)
    d["gpT"] = dp("gpT", [H, 2 * H], BF16, isOutput=False)
    d["hrT"] = dp("hrT", [H, NH], BF16, isOutput=False)
    d["hrb"] = dp("hrb", [NH], F32, isOutput=False)
    d["hcw"] = dp("hcw", [P, NH * 3 * 4], F32, isOutput=False)
    d["hcb"] = dp("hcb", [P, NH * 3], F32, isOutput=False)
    for nm in ("wqT", "wkT", "wvT", "woT"):
        d[nm] = dp(nm, [P, 4 * HD], BF16, isOutput=False)
    d["wgT"] = dp("wgT", [P, 4], BF16, isOutput=False)
    d["wgb"] = dp("wgb", [4], F32, isOutput=False)
    d["mgT"] = dp("mgT", [H, H], BF16, isOutput=False)
    d["mgb"] = dp("mgb", [P, HI], F32, isOutput=False)
    d["mxT"] = dp("mxT", [H, H], BF16, isOutput=False)
    d["mxb"] = dp("mxb", [P, HI], F32, isOutput=False)
    d["fiT"] = dp("fiT", [H, FI], BF16, isOutput=False)
    d["foT"] = dp("foT", [INNER, H], BF16, isOutput=False)
    for nm in ("g1w", "g2w", "g3w"):
        d[nm] = dp(nm, [H], BF16, isOutput=False)
    d["gb"] = dp("gb", [3], F32, isOutput=False)
    d["sel12"] = dp("sel12", [NH, NH * P], BF16, isOutput=False)
    d["outT"] = dp("outT", [H, TM], F32, isOutput=True)
    d["x1p_spill"] = nc.dram_tensor("x1p_spill", [P, HI * TM], BF16)
    d["mix_spill"] = nc.dram_tensor("mix_spill", [P, HI * TM], BF16)

    dump_shapes = dict(xh1=[H, T], hconv=[H, T], x1m=[H, TM], x1p=[H, TM],
                       xh=[H, T], hw=[NH, TM], memout=[4 * HD, TM],
                       mixin=[H, TM], x2=[H, TM], s3=[P, TM])
    dump_d = {nm: dp("dump_" + nm, dump_shapes[nm], F32, isOutput=True)
              for nm in dumps}

    with tile.TileContext(nc) as tc:
        _body(nc, tc, d, gelu_func, dump_d)
    nc.compile()
    return nc


def _body(nc, tc, d, gelu_func, dump_d):
    from contextlib import ExitStack

    def dump(name, tiles_fn, nchunk, ncols, prows=P):
        if name not in dump_d:
            return
        with tc.tile_pool(name=f"dmp{name}", bufs=2) as dpool:
            for ki in range(nchunk):
                t = dpool.tile([prows, ncols], F32, tag="d", name=f"d{name}")
                nc.vector.tensor_copy(t[:], tiles_fn(ki))
                nc.sync.dma_start(
                    out=dump_d[name].ap()[ki * prows:(ki + 1) * prows, :],
                    in_=t[:])

    def rms_xhat(pool, src_fn, nw, xhat, invr, ncols, tag):
        """src_fn(ki, lo, hi) -> AP [128, hi-lo] for a 512-aligned block.
        Writes xhat (bf16 [128, HI, ncols]) and invr [1, ncols] = sqrt(msq+eps)."""
        nb = ncols // 512
        t1 = pool.tile([P, ncols], F32, tag="t1", name=f"t1{tag}")
        rstd = pool.tile([P, ncols], F32, tag="rstd", name=f"rstd{tag}")
        for b in range(nb):
            xsq = pool.tile([P, 512], BF16, tag="xsq", bufs=2, name=f"xsq{tag}")
            ps = nc._pp_mm.tile([P, 512], F32, tag="mm", name=f"rms{tag}")
            for ki in range(HI):
                s = src_fn(ki, b * 512, (b + 1) * 512)
                nc.vector.tensor_tensor(xsq[:], s, s, op=OP.mult)
                nc.tensor.matmul(ps[:], nc._ones128[:], xsq[:],
                                 start=(ki == 0), stop=(ki == HI - 1))
            nc.vector.tensor_scalar(t1[:, b * 512:(b + 1) * 512], ps[:],
                                    1.0 / H, EPS, op0=OP.mult, op1=OP.add)
        nc.scalar.activation(invr[:], t1[0:1, :], AF.Sqrt)
        for b in range(nb):
            sl = slice(b * 512, (b + 1) * 512)
            scr = pool.tile([P, 512], F32, tag="scr", bufs=1, name=f"scr{tag}")
            nc.vector.reciprocal_approx_accurate(rstd[:, sl], t1[:, sl], scr[:])
        nc.scalar.activation(rstd[:], rstd[:], AF.Sqrt)
        for ki in range(HI):
            for b in range(nb):
                lo, hi = b * 512, (b + 1) * 512
                nc.vector.scalar_tensor_tensor(
                    xhat[:, ki, lo:hi], src_fn(ki, lo, hi), nw[:, ki:ki + 1],
                    rstd[:, lo:hi], op0=OP.mult, op1=OP.mult)

    def gate_row(pool, xhat, gw, gb_ap, invr, lo, hi, tag):
        """bf16 [128, hi-lo] broadcast of sigmoid((xhat^T gw)*invr + b) over
        token cols [lo, hi) of xhat."""
        gbc = pool.tile([P, hi - lo], BF16, tag=f"gbc{tag}", name=f"gbc{tag}")
        for b in range((hi - lo) // 512):
            sl = slice(lo + b * 512, lo + (b + 1) * 512)
            osl = slice(b * 512, (b + 1) * 512)
            ps = nc._pp_row.tile([1, 512], F32, tag="row", name=f"gr{tag}")
            for ki in range(HI):
                nc.tensor.matmul(ps[:], gw[:, ki:ki + 1], xhat[:, ki, sl],
                                 start=(ki == 0), stop=(ki == HI - 1))
            tmp = pool.tile([1, 512], F32, tag="gtmp", bufs=1, name=f"gt{tag}")
            nc.vector.tensor_tensor(tmp[:], ps[:], invr[:, sl], op=OP.mult)
            nc.scalar.activation(tmp[:], tmp[:], AF.Sigmoid, bias=gb_ap)
            ps2 = nc._pp_mm.tile([P, 512], F32, tag="mm", name=f"gb{tag}")
            nc.tensor.matmul(ps2[:], nc._ones1f[:], tmp[:], start=True,
                             stop=True)
            nc.vector.tensor_copy(gbc[:, osl], ps2[:])
        return gbc

    def gemm(wpool, w_dram, mo_list, rhs_fn, ncols, evict_fn, tag, k_chunks=HI):
        nb = ncols // 512
        for mo in mo_list:
            w = wpool.tile([P, k_chunks, P], BF16, tag="w", name=f"w{tag}")
            nc.sync.dma_start(out=w[:], in_=w_dram.ap()[:, mo * P:(mo + 1) * P]
                              .rearrange("(ki p) m -> p ki m", p=P))
            for b in range(nb):
                ps = nc._pp_mm.tile([P, 512], F32, tag="mm", name=f"mm{tag}")
                for ki in range(k_chunks):
                    nc.tensor.matmul(ps[:], w[:, ki], rhs_fn(ki, b),
                                     start=(ki == 0), stop=(ki == k_chunks - 1))
                evict_fn(mo, b, ps)

    with ExitStack() as ctx:
        persist = ctx.enter_context(tc.tile_pool(name="persist", bufs=1))
        pp_mm = ctx.enter_context(tc.tile_pool(name="pp_mm", bufs=3,
                                               space="PSUM"))
        pp_row = ctx.enter_context(tc.tile_pool(name="pp_row", bufs=2,
                                                space="PSUM"))
        nc._pp_mm, nc._pp_row = pp_mm, pp_row

        ones128 = persist.tile([P, P], BF16, tag="ones128")
        nc.vector.memset(ones128[:], 1.0)
        ones1 = persist.tile([1, P], BF16, tag="ones1")
        nc.vector.memset(ones1[:], 1.0)
        ones1f = persist.tile([1, P], F32, tag="ones1f")
        nc.vector.memset(ones1f[:], 1.0)
        nc._ones128, nc._ones1, nc._ones1f = ones128, ones1, ones1f

        def ld(pool, name, shape, dt, src_ap):
            t = pool.tile(shape, dt, tag=name, name=name + "_sb")
            nc.sync.dma_start(out=t[:], in_=src_ap)
            return t

        nw1 = ld(persist, "nw1", [P, HI], F32,
                 d["nw1"].ap().rearrange("(i p) -> p i", p=P))
        nw2 = ld(persist, "nw2", [P, HI], F32,
                 d["nw2"].ap().rearrange("(i p) -> p i", p=P))
        nw3 = ld(persist, "nw3", [P, HI], F32,
                 d["nw3"].ap().rearrange("(i p) -> p i", p=P))
        g1w = ld(persist, "g1w", [P, HI], BF16,
                 d["g1w"].ap().rearrange("(i p) -> p i", p=P))
        g2w = ld(persist, "g2w", [P, HI], BF16,
                 d["g2w"].ap().rearrange("(i p) -> p i", p=P))
        g3w = ld(persist, "g3w", [P, HI], BF16,
                 d["g3w"].ap().rearrange("(i p) -> p i", p=P))
        gb3 = ld(persist, "gb3", [1, 3], F32, d["gb"].ap()[None, :])

        # residual stream (main tokens), evolves in place through all stages
        xm = persist.tile([P, HI, TM], F32, tag="xm")
        nc.sync.dma_start(out=xm[:], in_=d["xT"].ap()[:, TM:]
                          .rearrange("(i p) t -> p i t", p=P))

        # ================= Stage 1 =================
        with ExitStack() as s1:
            spool = s1.enter_context(tc.tile_pool(name="s1misc", bufs=1))
            cpool = s1.enter_context(tc.tile_pool(name="s1conv", bufs=2))
            csw = ld(spool, "csw", [P, 6, HI, 4], F32,
                     d["csw"].ap().rearrange("p (l i k) -> p l i k",
                                             l=6, i=HI, k=4))
            csb = ld(spool, "csb", [P, 6, HI], F32,
                     d["csb"].ap().rearrange("p (l i) -> p l i", l=6, i=HI))
            cpb = ld(spool, "cpb", [P, HI], F32, d["cpb"].ap())
            invr1 = spool.tile([1, T], F32, tag="invr1")

            with tc.tile_pool(name="p_xp", bufs=1) as p_xp:
                xp = p_xp.tile([P, HI, TM], BF16, tag="xp")
                with tc.tile_pool(name="xpl", bufs=2) as xpl:
                    for ki in range(HI):
                        tmp = xpl.tile([P, TM], F32, tag="xpf")
                        nc.sync.dma_start(out=tmp[:], in_=d["xT"].ap()
                                          [ki * P:(ki + 1) * P, 0:TM])
                        nc.vector.tensor_copy(xp[:, ki], tmp[:])

                def src1(ki, lo, hi):
                    return xp[:, ki, lo:hi] if hi <= TM \
                        else xm[:, ki, lo - TM:hi - TM]

                xh1 = cpool.tile([P, HI, T], BF16, tag="cb", name="xh1")
                with tc.tile_pool(name="s1rms", bufs=1) as rpool:
                    rms_xhat(rpool, src1, nw1, xh1, invr1, T, "r1")
                dump("xh1", lambda ki: xh1[:, ki], HI, T)
                g1bc = gate_row(spool, xh1, g1w, gb3[:, 0:1], invr1, 0, T, "g1")

                src_t = xh1
                for li, dl in enumerate(STACK_DILS):
                    dst = cpool.tile([P, HI, T], BF16, tag="cb", name=f"cs{li}")
                    for ki in range(HI):
                        nc.vector.tensor_scalar(
                            dst[:, ki], src_t[:, ki], csw[:, li, ki, 3:4],
                            csb[:, li, ki:ki + 1], op0=OP.mult, op1=OP.add)
                        for k in (2, 1, 0):
                            s = (3 - k) * dl
                            nc.vector.scalar_tensor_tensor(
                                dst[:, ki, s:T], src_t[:, ki, 0:T - s],
                                csw[:, li, ki, k:k + 1], dst[:, ki, s:T],
                                op0=OP.mult, op1=OP.add)
                    nc.scalar.activation(dst[:, :, :], dst[:, :, :], gelu_func)
                    nc.gpsimd.tensor_tensor(dst[:, :, :], src_t[:, :, :],
                                            dst[:, :, :], op=OP.add)
                    src_t = dst
                hconv = src_t
                dump("hconv", lambda ki: hconv[:, ki], HI, T)

                # conv_proj fused with x1 update; prefix spilled to DRAM
                with tc.tile_pool(name="s1w", bufs=3) as wpool, \
                     tc.tile_pool(name="s1e", bufs=4) as epool:
                    def ev(mo, b, ps):
                        sl = slice(b * 512, (b + 1) * 512)
                        y = epool.tile([P, 512], BF16, tag="y1", name="y1t")
                        nc.scalar.activation(y[:], ps[:], AF.Identity,
                                             bias=cpb[:, mo:mo + 1])
                        gy = epool.tile([P, 512], BF16, tag="gy", name="gy1t")
                        nc.vector.tensor_tensor(gy[:], y[:], g1bc[:, sl],
                                                op=OP.mult)
                        if b < 2:
                            x1p = epool.tile([P, 512], BF16, tag="x1p",
                                             name="x1pt")
                            nc.gpsimd.tensor_tensor(x1p[:], xp[:, mo, sl],
                                                    gy[:], op=OP.add)
                            nc.sync.dma_start(
                                out=d["x1p_spill"].ap()
                                [:, mo * TM + b * 512:mo * TM + (b + 1) * 512],
                                in_=x1p[:])
                        else:
                            slm = slice((b - 2) * 512, (b - 1) * 512)
                            nc.vector.tensor_tensor(xm[:, mo, slm],
                                                    xm[:, mo, slm], gy[:],
                                                    op=OP.add)

                    gemm(wpool, d["cpT"], range(HI),
                         lambda ki, b: hconv[:, ki, b * 512:(b + 1) * 512],
                         T, ev, "cp")
            dump("x1m", lambda ki: xm[:, ki], HI, TM)

        # ================= Stage 2 =================
        with tc.tile_pool(name="p_g2m", bufs=1) as p_mix:
            with tc.tile_pool(name="p_xh", bufs=1) as p_xh:
                # ---- 2a: rms2 / gate_proj / router / g2
                with ExitStack() as s2a:
                    spool = s2a.enter_context(tc.tile_pool(name="s2a", bufs=1))
                    hrT = ld(spool, "hrT", [P, HI, NH], BF16,
                             d["hrT"].ap().rearrange("(ki p) m -> p ki m", p=P))
                    hrb = ld(spool, "hrb", [NH, 1], F32, d["hrb"].ap()[:, None])
                    invr2 = spool.tile([1, T], F32, tag="invr2")
                    xh2 = spool.tile([P, HI, T], BF16, tag="xh2")
                    with tc.tile_pool(name="s2x1p", bufs=3) as xppool:
                        def src2(ki, lo, hi):
                            if hi <= TM:
                                blk = xppool.tile([P, 512], BF16, tag="xpb",
                                                  name="x1pblk")
                                nc.sync.dma_start(
                                    out=blk[:], in_=d["x1p_spill"].ap()
                                    [:, ki * TM + lo:ki * TM + hi])
                                return blk[:]
                            return xm[:, ki, lo - TM:hi - TM]
                        with tc.tile_pool(name="s2rms", bufs=1) as rpool:
                            rms_xhat(rpool, src2, nw2, xh2, invr2, T, "r2")
                    g2m = p_mix.tile([P, TM], BF16, tag="g2m")
                    g2bc = gate_row(spool, xh2, g2w, gb3[:, 1:2], invr2,
                                    TM, T, "g2")
                    nc.vector.tensor_copy(g2m[:], g2bc[:])

                    xh = p_xh.tile([P, NH, T], BF16, tag="xh")
                    hwr = p_xh.tile([NH, TM], F32, tag="hwr")
                    with tc.tile_pool(name="s2w", bufs=3) as wpool, \
                         tc.tile_pool(name="s2e", bufs=4) as epool:
                        sig = {}

                        def ev_g(mo, b, ps):
                            t_ = epool.tile([P, 512], BF16, tag="sg",
                                            name="sigg")
                            nc.scalar.activation(t_[:], ps[:], AF.Sigmoid)
                            sig[b] = t_

                        def mk_ev_a(mo_head):
                            def ev_a(mo, b, ps):
                                t_ = epool.tile([P, 512], BF16, tag="ar",
                                                name="araw")
                                nc.vector.tensor_copy(t_[:], ps[:])
                                nc.vector.tensor_tensor(
                                    xh[:, mo_head, b * 512:(b + 1) * 512],
                                    t_[:], sig[b][:], op=OP.mult)
                            return ev_a

                        def rhs2(ki, b):
                            return xh2[:, ki, b * 512:(b + 1) * 512]
                        for mo in range(HI):
                            gemm(wpool, d["gpT"], [HI + mo], rhs2, T, ev_g,
                                 "gp")
                            gemm(wpool, d["gpT"], [mo], rhs2, T, mk_ev_a(mo),
                                 "gp")
                    dump("xh", lambda ki: xh[:, ki], NH, T)

                    for b in range(NBM):
                        ps = nc._pp_row.tile([NH, 512], F32, tag="row",
                                             name="hwps")
                        for ki in range(HI):
                            nc.tensor.matmul(
                                ps[:], hrT[:, ki],
                                xh2[:, ki, TM + b * 512:TM + (b + 1) * 512],
                                start=(ki == 0), stop=(ki == HI - 1))
                        nc.scalar.activation(hwr[:, b * 512:(b + 1) * 512],
                                             ps[:], AF.Sigmoid, bias=hrb[:])
                    dump("hw", lambda _: hwr[:, :], 1, TM, prows=NH)

                # ---- 2b: memory + head convs -> mix_in (spilled to DRAM)
                _stage2b(nc, tc, d, dump, ld, xh, hwr)
            # ---- 2c: mix gate + mixing -> x2 (xh freed)
            _stage2c(nc, tc, d, dump, ld, gemm, xm, g2m)

        # ================= Stage 3 =================
        _stage3(nc, tc, d, dump, ld, gemm, gate_row, rms_xhat, xm, nw3, g3w,
                gb3)

        nc.sync.dma_start(out=d["outT"].ap().rearrange("(i p) t -> p i t", p=P),
                          in_=xm[:])


def _stage2b(nc, tc, d, dump, ld, xh, hwr):
    from contextlib import ExitStack
    with ExitStack() as s2:
        spool = s2.enter_context(tc.tile_pool(name="s2b", bufs=1))
        hcw = ld(spool, "hcw", [P, NH, 3, 4], F32,
                 d["hcw"].ap().rearrange("p (i l k) -> p i l k", i=NH, l=3,
                                         k=4))
        hcb = ld(spool, "hcb", [P, NH, 3], F32,
                 d["hcb"].ap().rearrange("p (i l) -> p i l", i=NH, l=3))
        wqT = ld(spool, "wqT", [P, 4, HD], BF16,
                 d["wqT"].ap().rearrange("p (m e) -> p m e", m=4))
        wkT = ld(spool, "wkT", [P, 4, HD], BF16,
                 d["wkT"].ap().rearrange("p (m e) -> p m e", m=4))
        wvT = ld(spool, "wvT", [P, 4, MEM], BF16,
                 d["wvT"].ap().rearrange("p (m e) -> p m e", m=4))
        woT = ld(spool, "woT", [P, 4, HD], BF16,
                 d["woT"].ap().rearrange("p (m e) -> p m e", m=4))
        wgT = ld(spool, "wgT", [P, 4], BF16, d["wgT"].ap())
        wgbr = ld(spool, "wgbr", [1, 4], F32, d["wgb"].ap()[None, :])
        sel12 = ld(spool, "sel12", [NH, NH, P], BF16,
                   d["sel12"].ap().rearrange("k (i m) -> k i m", i=NH))
        wgb64 = spool.tile([64, 4], F32, tag="wgb64")
        psb = nc._pp_row.tile([64, 4], F32, tag="row", name="wgb64ps")
        nc.tensor.matmul(psb[:], nc._ones1f[:, 0:64], wgbr[:], start=True,
                         stop=True)
        nc.vector.tensor_copy(wgb64[:], psb[:])

        M = spool.tile([P, 4, HD], F32, tag="Msb")
        nc.vector.memset(M[:], 0.0)
        readsT = spool.tile([P, 4, TM], BF16, tag="readsT")
        sgT = spool.tile([64, 4, NCH], F32, tag="sgT")
        sgTb = spool.tile([64, 4, NCH], BF16, tag="sgTb")
        decayb = spool.tile([P, 4, NCH], F32, tag="decayb")
        ones64 = spool.tile([64, 1], BF16, tag="ones64")
        nc.vector.memset(ones64[:], 1.0)

        for m in range(4):
            hd = MEM_HEADS[m]
            ps = nc._pp_row.tile([64, NCH], F32, tag="row", name="gps")
            for c in range(NCH):
                nc.tensor.matmul(ps[:, c:c + 1], xh[:, hd, c * ST:(c + 1) * ST],
                                 wgT[:, m:m + 1], start=True, stop=True)
            nc.scalar.activation(sgT[:, m], ps[:], AF.Sigmoid,
                                 bias=wgb64[:, m:m + 1])
            nc.vector.tensor_copy(sgTb[:, m], sgT[:, m])
            ps2 = nc._pp_row.tile([1, NCH], F32, tag="row", name="dps")
            nc.tensor.matmul(ps2[:], ones64[:], sgTb[:, m], start=True,
                             stop=True)
            drow = spool.tile([1, NCH], BF16, tag="drow", bufs=2, name="drowt")
            nc.vector.tensor_scalar(drow[:], ps2[:], -1.0 / ST, 1.0,
                                    op0=OP.mult, op1=OP.add)
            ps3 = nc._pp_mm.tile([P, NCH], F32, tag="mm", name="dbps")
            nc.tensor.matmul(ps3[:], nc._ones1[:], drow[:], start=True,
                             stop=True)
            nc.vector.tensor_copy(decayb[:, m], ps3[:])

        with tc.tile_pool(name="qblk", bufs=1) as qpool, \
             tc.tile_pool(name="kvblk", bufs=6) as kvpool, \
             tc.tile_pool(name="pps", bufs=3, space="PSUM") as pps:
            for b in range(NB):
                qb = qpool.tile([P, 4, 512], F32, tag="qb")
                for m in range(4):
                    ps = pps.tile([P, 512], F32, tag="scan", name="qps")
                    nc.tensor.matmul(ps[:], wqT[:, m],
                                     xh[:, MEM_HEADS[m], b * 512:(b + 1) * 512],
                                     start=True, stop=True)
                    nc.vector.tensor_copy(qb[:, m], ps[:])
                for cc in range(8):
                    c = b * 8 + cc
                    for m in range(4):
                        hd = MEM_HEADS[m]
                        xm_c = xh[:, hd, c * ST:(c + 1) * ST]
                        psk = pps.tile([64, HD], F32, tag="scan", name="kps")
                        nc.tensor.matmul(psk[:], xm_c, wkT[:, m], start=True,
                                         stop=True)
                        kts = kvpool.tile([64, HD], BF16, tag="kts")
                        nc.scalar.activation(kts[:], psk[:], AF.Copy,
                                             scale=sgT[:, m, c:c + 1])
                        psv = pps.tile([64, MEM], F32, tag="scan", name="vps")
                        nc.tensor.matmul(psv[:], xm_c, wvT[:, m], start=True,
                                         stop=True)
                        vts = kvpool.tile([64, MEM], BF16, tag="vts")
                        nc.vector.tensor_copy(vts[:], psv[:])
                        if c >= 16:
                            psr = pps.tile([P, ST], F32, tag="scan", name="rps")
                            nc.tensor.matmul(psr[:], M[:, m],
                                             qb[:, m, cc * ST:(cc + 1) * ST],
                                             start=True, stop=True)
                            nc.vector.tensor_copy(
                                readsT[:, m, (c - 16) * ST:(c - 15) * ST],
                                psr[:])
                        psw = pps.tile([P, MEM], F32, tag="scan", name="wps")
                        nc.tensor.matmul(psw[:], kts[:], vts[:], start=True,
                                         stop=True)
                        nc.vector.scalar_tensor_tensor(
                            M[:, m], M[:, m], decayb[:, m, c:c + 1], psw[:],
                            op0=OP.mult, op1=OP.add)

        mix_in = spool.tile([P, HI, TM], BF16, tag="mixin")
        hwrb = spool.tile([NH, TM], BF16, tag="hwrb")
        nc.vector.tensor_copy(hwrb[:], hwr[:])
        memy = spool.tile([P, 4, TM], BF16, tag="memy")
        for m in range(4):
            for b in range(NBM):
                ps = nc._pp_mm.tile([P, 512], F32, tag="mm", name="mops")
                nc.tensor.matmul(ps[:], woT[:, m],
                                 readsT[:, m, b * 512:(b + 1) * 512],
                                 start=True, stop=True)
                nc.vector.tensor_copy(memy[:, m, b * 512:(b + 1) * 512], ps[:])
        dump("memout", lambda m: memy[:, m], 4, TM)

        with tc.tile_pool(name="hcp", bufs=3) as hcp:
            for i in range(NH):
                src_t = xh[:, i]
                for li in range(3):
                    dl = HEAD_DILS[i][li]
                    dst = hcp.tile([P, T], BF16, tag="hc", name=f"hc{i}_{li}")
                    nc.vector.tensor_scalar(dst[:], src_t, hcw[:, i, li, 3:4],
                                            hcb[:, i, li:li + 1],
                                            op0=OP.mult, op1=OP.add)
                    nc.gpsimd.tensor_tensor(dst[:], dst[:], src_t, op=OP.add)
                    for k in (2, 1, 0):
                        s = (3 - k) * dl
                        if s >= T:
                            continue
                        nc.vector.scalar_tensor_tensor(
                            dst[:, s:T], src_t[:, 0:T - s],
                            hcw[:, i, li, k:k + 1], dst[:, s:T],
                            op0=OP.mult, op1=OP.add)
                    src_t = dst
                if i in MEM_HEADS:
                    m = i - MEM_HEADS[0]
                    nc.vector.tensor_tensor(src_t[:, TM:T], src_t[:, TM:T],
                                            memy[:, m], op=OP.add)
                for b in range(NBM):
                    sl = slice(b * 512, (b + 1) * 512)
                    ps = nc._pp_mm.tile([P, 512], F32, tag="mm", name="hwb")
                    nc.tensor.matmul(ps[:], sel12[:, i], hwrb[:, sl],
                                     start=True, stop=True)
                    nc.vector.tensor_tensor(
                        mix_in[:, i, sl],
                        src_t[:, TM + b * 512:TM + (b + 1) * 512], ps[:],
                        op=OP.mult)
        dump("mixin", lambda ki: mix_in[:, ki], HI, TM)
        nc.sync.dma_start(out=d["mix_spill"].ap()
                          .rearrange("p (i t) -> p i t", i=HI), in_=mix_in[:])


def _stage2c(nc, tc, d, dump, ld, gemm, xm, g2m):
    from contextlib import ExitStack
    with ExitStack() as s2:
        spool = s2.enter_context(tc.tile_pool(name="s2c", bufs=1))
        mix_in = spool.tile([P, HI, TM], BF16, tag="mixin2")
        nc.sync.dma_start(out=mix_in[:], in_=d["mix_spill"].ap()
                          .rearrange("p (i t) -> p i t", i=HI))
        mgb = ld(spool, "mgb", [P, HI], F32, d["mgb"].ap())
        mxb = ld(spool, "mxb", [P, HI], F32, d["mxb"].ap())
        s2t = spool.tile([P, HI, TM], BF16, tag="s2t")
        y2 = spool.tile([P, HI, TM], BF16, tag="y2")
        with tc.tile_pool(name="s2cw", bufs=3) as wpool, \
             tc.tile_pool(name="s2ce", bufs=4) as epool:
            def ev_mg(mo, b, ps):
                sl = slice(b * 512, (b + 1) * 512)
                t_ = epool.tile([P, 512], BF16, tag="smg", name="sigmg")
                nc.scalar.activation(t_[:], ps[:], AF.Sigmoid,
                                     bias=mgb[:, mo:mo + 1])
                nc.vector.tensor_tensor(s2t[:, mo, sl], mix_in[:, mo, sl],
                                        t_[:], op=OP.mult)

            gemm(wpool, d["mgT"], range(HI),
                 lambda ki, b: mix_in[:, ki, b * 512:(b + 1) * 512],
                 TM, ev_mg, "mg")

            def ev_mx(mo, b, ps):
                sl = slice(b * 512, (b + 1) * 512)
                t_ = epool.tile([P, 512], BF16, tag="y2e", name="y2e")
                nc.scalar.activation(t_[:], ps[:], AF.Identity,
                                     bias=mxb[:, mo:mo + 1])
                nc.vector.tensor_tensor(y2[:, mo, sl], t_[:], g2m[:, sl],
                                        op=OP.mult)

            gemm(wpool, d["mxT"], range(HI),
                 lambda ki, b: s2t[:, ki, b * 512:(b + 1) * 512],
                 TM, ev_mx, "mx")

        for ki in range(HI):
            nc.vector.tensor_tensor(xm[:, ki], xm[:, ki], y2[:, ki], op=OP.add)
        dump("x2", lambda ki: xm[:, ki], HI, TM)


def _stage3(nc, tc, d, dump, ld, gemm, gate_row, rms_xhat, xm, nw3, g3w, gb3):
    from contextlib import ExitStack
    with ExitStack() as s3:
        spool = s3.enter_context(tc.tile_pool(name="s3", bufs=1))
        invr3 = spool.tile([1, TM], F32, tag="invr3")
        xh3 = spool.tile([P, HI, TM], BF16, tag="xh3")
        with tc.tile_pool(name="s3rms", bufs=1) as rpool:
            rms_xhat(rpool, lambda ki, lo, hi: xm[:, ki, lo:hi], nw3, xh3,
                     invr3, TM, "r3")
        g3bc = gate_row(spool, xh3, g3w, gb3[:, 2:3], invr3, 0, TM, "g3")

        s3t = spool.tile([P, 48, TM], BF16, tag="s3t")
        with tc.tile_pool(name="s3w", bufs=3) as wpool, \
             tc.tile_pool(name="s3e", bufs=4) as epool:
            sig = {}

            def ev_fg(mo, b, ps):
                t_ = epool.tile([P, 512], BF16, tag="sf", name="sigf")
                nc.scalar.activation(t_[:], ps[:], AF.Sigmoid)
                sig[b] = t_

            def mk_ev_fc(co):
                def ev_fc(mo, b, ps):
                    sl = slice(b * 512, (b + 1) * 512)
                    t_ = epool.tile([P, 512], BF16, tag="cr", name="craw")
                    nc.vector.tensor_copy(t_[:], ps[:])
                    nc.vector.tensor_tensor(s3t[:, co, sl], t_[:], sig[b],
                                            op=OP.mult)
                return ev_fc

            def rhs3(ki, b):
                return xh3[:, ki, b * 512:(b + 1) * 512]
            for co in range(48):
                gemm(wpool, d["fiT"], [48 + co], rhs3, TM, ev_fg, "fi")
                gemm(wpool, d["fiT"], [co], rhs3, TM, mk_ev_fc(co), "fi")
        dump("s3", lambda ki: s3t[:, 0], 1, TM)

        with tc.tile_pool(name="s3wo", bufs=2) as wpool, \
             tc.tile_pool(name="s3oe", bufs=4) as epool:
            for mo in range(HI):
                w = wpool.tile([P, 48, P], BF16, tag="wo", name="wfo")
                nc.sync.dma_start(out=w[:], in_=d["foT"].ap()
                                  [:, mo * P:(mo + 1) * P]
                                  .rearrange("(ki p) m -> p ki m", p=P))
                for b in range(NBM):
                    sl = slice(b * 512, (b + 1) * 512)
                    ps = nc._pp_mm.tile([P, 512], F32, tag="mm", name="mmfo")
                    for ki in range(48):
                        nc.tensor.matmul(ps[:], w[:, ki], s3t[:, ki, sl],
                                         start=(ki == 0), stop=(ki == 47))
                    t_ = epool.tile([P, 512], BF16, tag="gy3", name="gy3")
                    nc.vector.tensor_tensor(t_[:], ps[:], g3bc[:, sl],
                                            op=OP.mult)
                    nc.vector.tensor_tensor(xm[:, mo, sl], xm[:, mo, sl],
                                            t_[:], op=OP.add)


# ----------------------------------------------------------------------------
# host side
# ----------------------------------------------------------------------------

_CACHE = {}


def _prep_weights(inputs):
    f = lambda a: np.ascontiguousarray(np.asarray(a, dtype=np.float32))
    bf = lambda a: np.ascontiguousarray(np.asarray(a, np.float32).astype(BF))
    w = {}
    w["nw1"] = f(inputs["norm1_w"])
    w["nw2"] = f(inputs["norm2_w"])
    w["nw3"] = f(inputs["norm3_w"])
    w["csw"] = f(np.asarray(inputs["convstack_w"]).reshape(6, HI, P, 4)
                 .transpose(2, 0, 1, 3).reshape(P, 6 * HI * 4))
    w["csb"] = f(np.asarray(inputs["convstack_b"]).reshape(6, HI, P)
                 .transpose(2, 0, 1).reshape(P, 6 * HI))
    w["cpT"] = bf(np.asarray(inputs["conv_proj_w"]).T)
    w["cpb"] = f(np.asarray(inputs["conv_proj_b"]).reshape(HI, P).T)
    w["gpT"] = bf(np.asarray(inputs["gate_proj_w"]).T)
    w["hrT"] = bf(np.asarray(inputs["head_router_w"]).T)
    w["hrb"] = f(inputs["head_router_b"])
    w["hcw"] = f(np.asarray(inputs["head_conv_w"]).transpose(2, 0, 1, 3)
                 .reshape(P, NH * 3 * 4))
    w["hcb"] = f(np.asarray(inputs["head_conv_b"]).transpose(2, 0, 1)
                 .reshape(P, NH * 3))
    w["wqT"] = bf(np.asarray(inputs["mem_Wq"]).transpose(2, 0, 1)
                  .reshape(P, 4 * HD))
    w["wkT"] = bf(np.asarray(inputs["mem_Wk"]).transpose(2, 0, 1)
                  .reshape(P, 4 * HD))
    w["wvT"] = bf(np.asarray(inputs["mem_Wv"]).transpose(2, 0, 1)
                  .reshape(P, 4 * MEM))
    w["woT"] = bf(np.asarray(inputs["mem_Wout"]).transpose(2, 0, 1)
                  .reshape(P, 4 * HD))
    w["wgT"] = bf(np.asarray(inputs["mem_Wg_w"]).T)
    w["wgb"] = f(inputs["mem_Wg_b"])
    w["mgT"] = bf(np.asarray(inputs["mix_gate_w"]).T)
    w["mgb"] = f(np.asarray(inputs["mix_gate_b"]).reshape(HI, P).T)
    w["mxT"] = bf(np.asarray(inputs["mixing_w"]).T)
    w["mxb"] = f(np.asarray(inputs["mixing_b"]).reshape(HI, P).T)
    w["fiT"] = bf(np.asarray(inputs["ffn_in_w"]).T)
    w["foT"] = bf(np.asarray(inputs["ffn_out_w"]).T)
    nw = [inputs["norm1_w"], inputs["norm2_w"], inputs["norm3_w"]]
    gw = [inputs["conv_gate_w"], inputs["state_gate_w"], inputs["ffn_gate_w"]]
    for i, nm in enumerate(["g1w", "g2w", "g3w"]):
        w[nm] = bf(np.asarray(gw[i], np.float32) / np.asarray(nw[i], np.float32))
    w["gb"] = f(np.array([np.asarray(inputs["conv_gate_b"]).ravel()[0],
                          np.asarray(inputs["state_gate_b"]).ravel()[0],
                          np.asarray(inputs["ffn_gate_b"]).ravel()[0]]))
    sel = np.zeros((NH, NH, P), np.float32)
    for i in range(NH):
        sel[i, i, :] = 1.0
    w["sel12"] = bf(sel.reshape(NH, NH * P))
    return w


def make_in_maps(inputs):
    w = _prep_weights(inputs)
    x = np.asarray(inputs["x"], np.float32)
    in_maps = []
    for c in range(8):
        b, half = c // 2, c % 2
        xT = np.ascontiguousarray(x[b].T)
        if half == 0:
            xc = np.concatenate([np.zeros((H, TM), np.float32), xT[:, :TM]],
                                axis=1)
        else:
            xc = xT
        m = dict(w)
        m["xT"] = xc
        in_maps.append(m)
    return in_maps


def assemble_output(results):
    out = np.empty((4, T, H), np.float32)
    for c in range(8):
        b, half = c // 2, c % 2
        out[b, half * TM:(half + 1) * TM, :] = results[c]["outT"].T
    return out


def kernel(**inputs):
    if "nc" not in _CACHE:
        _CACHE["nc"] = build_program()
    nc = _CACHE["nc"]
    in_maps = make_in_maps(inputs)
    res = run_bass_kernel_spmd(nc, in_maps, core_ids=list(range(8)))
    return assemble_output(res.results)
